# revision 2
# baseline (speedup 1.0000x reference)
"""Trainium2 Bass kernel for: softmax(cid_time[current][:, history], axis=-1).

Shapes (hardcoded): history (4096,) int64, current (4096,) int64,
cid_time (10000, 10000) float32 -> out (4096, 4096) float32.

Strategy (8 NeuronCores, SPMD; table replicated, state_len sharded):
Each core owns 512 output rows.  Per core:
  A. indirect-DMA row-gather R = table[current_block]  (40KB contiguous rows)
  B. PE-transpose R into HBM staging S3 laid out [rb, t%128, t//128, i]
     so a table-column t of the gathered block is one contiguous 512B row
     (per rb sub-block of 128 i's), written at 40KB/partition line rate.
  C. indirect-DMA row-gather by history (host-transformed indices) ->
     E^T tiles [128 j, 512 i]
  D. exp on ScalarE; column-sums via ones-matmul accumulated in PSUM
  E. PE-transpose back to [i, j] with the softmax normalization fused into
     the PSUM->SBUF evacuation (per-partition reciprocal scalar), then
     contiguous DMA of the [512, 4096] output block.
Softmax skips max-subtraction: inputs are N(0,1) so exp is well-conditioned.
"""

import sys

if "/opt/trn_rl_repo" not in sys.path:
    sys.path.insert(0, "/opt/trn_rl_repo")

import numpy as np

N_CORES = 8
P = 128
NCID = 10000
NTIME = 10000
SEQ = 4096
STATE = 4096
ROWS = STATE // N_CORES          # 512 output rows per core
RB = ROWS // P                   # 4 row-tiles per core
TB = (NTIME + P - 1) // P        # 79 column (time) blocks
TB_LAST_W = NTIME - (TB - 1) * P # 16 wide last block
JB = SEQ // P                    # 32 j-tiles
JQ = JB // 4                     # 8 groups of 4 j-tiles

_prog_cache = {}


def _build_program():
    from concourse import bass, bacc, mybir
    import concourse.tile as tile
    from concourse.masks import make_identity

    f32 = mybir.dt.float32
    i32 = mybir.dt.int32
    Exp = mybir.ActivationFunctionType.Exp

    nc = bacc.Bacc("TRN2", target_bir_lowering=False, debug=False,
                   num_devices=N_CORES)
    table = nc.dram_tensor("table", [NCID, NTIME], f32, kind="ExternalInput").ap()
    cur = nc.dram_tensor("cur", [P, RB], i32, kind="ExternalInput").ap()
    hist = nc.dram_tensor("hist", [P, JB, RB], i32, kind="ExternalInput").ap()
    outp = nc.dram_tensor("outp", [ROWS, SEQ], f32, kind="ExternalOutput").ap()

    with tile.TileContext(nc) as tc:
        with tc.tile_pool(name="dram", bufs=1, space="DRAM") as dpool, \
             tc.tile_pool(name="const", bufs=1) as cpool:
            S3 = dpool.tile([RB, P, TB, P], f32)  # ~20.7MB HBM staging
            S3_flat = S3[:].rearrange("a b c d -> (a b c) d")

            ident = cpool.tile([P, P], f32)
            make_identity(nc, ident[:])
            ones = cpool.tile([P, 1], f32)
            nc.vector.memset(ones[:], 1.0)
            cur_sb = cpool.tile([P, RB], i32)
            nc.sync.dma_start(out=cur_sb[:], in_=cur[:, :])
            hist_sb = cpool.tile([P, JB, RB], i32)
            nc.sync.dma_start(out=hist_sb[:], in_=hist[:, :, :])
            sums_sb = cpool.tile([1, ROWS], f32)
            recip_sb = cpool.tile([P, RB], f32)

            # ---- Phase A/B: gather rows, transpose into S3 staging ----
            with tc.tile_pool(name="rpool", bufs=2) as rpool, \
                 tc.tile_pool(name="spool", bufs=2) as spool, \
                 tc.tile_pool(name="ppool", bufs=4, space="PSUM") as ppool:
                for rb in range(RB):
                    R = rpool.tile([P, NTIME], f32)
                    nc.gpsimd.indirect_dma_start(
                        out=R[:], out_offset=None,
                        in_=table[:, :],
                        in_offset=bass.IndirectOffsetOnAxis(
                            ap=cur_sb[:, rb:rb + 1], axis=0),
                    )
                    S3sb = spool.tile([P, TB, P], f32)
                    for tq in range((TB + 3) // 4):
                        ng = min(4, TB - tq * 4)
                        pt = ppool.tile([P, 512], f32)
                        if tq * 4 + ng == TB:
                            # last group holds the 16-wide partial block:
                            # zero so untouched partitions stay finite
                            nc.vector.memset(pt[:, (ng - 1) * P:ng * P], 0.0)
                        for u in range(ng):
                            tb = tq * 4 + u
                            w = P if tb < TB - 1 else TB_LAST_W
                            nc.tensor.transpose(
                                out=pt[:w, u * P:(u + 1) * P],
                                in_=R[:, tb * P:tb * P + w],
                                identity=ident[:],
                            )
                        eng = nc.vector if tq % 2 == 0 else nc.scalar
                        if eng is nc.vector:
                            eng.tensor_copy(out=S3sb[:, tq * 4:tq * 4 + ng, :],
                                            in_=pt[:, :ng * P])
                        else:
                            eng.copy(out=S3sb[:, tq * 4:tq * 4 + ng, :],
                                     in_=pt[:, :ng * P])
                    nc.sync.dma_start(out=S3[rb], in_=S3sb[:])

            # ---- Phase C/D/E ----
            with tc.tile_pool(name="xpool", bufs=1) as xpool, \
                 tc.tile_pool(name="epool", bufs=6) as epool, \
                 tc.tile_pool(name="opool", bufs=1) as opool, \
                 tc.tile_pool(name="psums", bufs=1, space="PSUM") as pspool, \
                 tc.tile_pool(name="prec", bufs=1, space="PSUM") as precpool, \
                 tc.tile_pool(name="p2", bufs=4, space="PSUM") as p2pool:
                X = xpool.tile([P, JB, RB * P], f32)      # 64KB/partition
                Osb = [opool.tile([P, SEQ], f32, tag=f"o{ib}", name=f"o{ib}")
                       for ib in range(RB)]               # 4 x 16KB/partition
                sums_ps = pspool.tile([1, ROWS], f32)

                for jb in range(JB):
                    E = epool.tile([P, RB * P], f32)
                    for rb in range(RB):
                        nc.gpsimd.indirect_dma_start(
                            out=E[:, rb * P:(rb + 1) * P], out_offset=None,
                            in_=S3_flat[:, :],
                            in_offset=bass.IndirectOffsetOnAxis(
                                ap=hist_sb[:, jb, rb:rb + 1], axis=0),
                        )
                    nc.scalar.activation(out=X[:, jb, :], in_=E[:], func=Exp)
                    nc.tensor.matmul(
                        out=sums_ps[:1, :], lhsT=ones[:, :1], rhs=X[:, jb, :],
                        start=(jb == 0), stop=(jb == JB - 1),
                        skip_group_check=True,
                    )

                # reciprocal of column sums, transposed to per-partition form
                nc.vector.tensor_copy(out=sums_sb[:1, :], in_=sums_ps[:1, :])
                rec_ps = precpool.tile([P, RB], f32)
                for ib in range(RB):
                    nc.tensor.transpose(
                        out=rec_ps[:, ib:ib + 1],
                        in_=sums_sb[:1, ib * P:(ib + 1) * P],
                        identity=ident[:1, :1],
                    )
                nc.vector.reciprocal(out=recip_sb[:, :], in_=rec_ps[:, :])

                # transpose back with fused normalize, then store
                for ib in range(RB):
                    for jq in range(JQ):
                        pt2 = p2pool.tile([P, 512], f32)
                        for u in range(4):
                            jb = jq * 4 + u
                            nc.tensor.transpose(
                                out=pt2[:, u * P:(u + 1) * P],
                                in_=X[:, jb, ib * P:(ib + 1) * P],
                                identity=ident[:],
                            )
                        if jq % 2 == 0:
                            nc.vector.tensor_scalar_mul(
                                out=Osb[ib][:, jq * 512:(jq + 1) * 512],
                                in0=pt2[:], scalar1=recip_sb[:, ib:ib + 1])
                        else:
                            nc.scalar.mul(
                                out=Osb[ib][:, jq * 512:(jq + 1) * 512],
                                in_=pt2[:], mul=recip_sb[:, ib:ib + 1])
                    nc.sync.dma_start(out=outp[ib * P:(ib + 1) * P, :],
                                      in_=Osb[ib][:])

    nc.compile()
    return nc


def _get_program():
    if "nc" not in _prog_cache:
        _prog_cache["nc"] = _build_program()
    return _prog_cache["nc"]


def _make_in_maps(history, current, cid_time):
    hist = np.asarray(history).astype(np.int64).ravel()
    curr = np.asarray(current).astype(np.int64).ravel()
    tab = np.ascontiguousarray(np.asarray(cid_time, dtype=np.float32))
    assert hist.shape == (SEQ,) and curr.shape == (STATE,)
    assert tab.shape == (NCID, NTIME)

    # history index j=jb*128+p -> S3_flat row for sub-block rb:
    #   rb*(128*TB) + (h%128)*TB + h//128
    h = hist
    base = (h % P).astype(np.int64) * TB + (h // P)
    hist_idx = np.empty((P, JB, RB), dtype=np.int32)
    for rb in range(RB):
        v = (rb * (P * TB) + base).astype(np.int32).reshape(JB, P)  # [jb, p]
        hist_idx[:, :, rb] = v.T

    in_maps = []
    for c in range(N_CORES):
        cur_block = curr[c * ROWS:(c + 1) * ROWS].astype(np.int32)
        cur_idx = cur_block.reshape(RB, P).T.copy()  # [p, rb]
        in_maps.append({
            "table": tab,
            "cur": cur_idx,
            "hist": hist_idx,
        })
    return in_maps


def run(history, current, cid_time, trace=False, **trace_kwargs):
    from concourse.bass_utils import run_bass_kernel_spmd

    nc = _get_program()
    in_maps = _make_in_maps(history, current, cid_time)
    res = run_bass_kernel_spmd(nc, in_maps, list(range(N_CORES)),
                               trace=trace, **trace_kwargs)
    out = np.concatenate([res.results[c]["outp"] for c in range(N_CORES)],
                         axis=0)
    return out, res


def kernel(history, current, cid_time):
    out, _ = run(history, current, cid_time, trace=False)
    return out


# revision 5
# speedup vs baseline: 1.4582x; 1.4582x over previous
"""Trainium2 Bass kernel for: softmax(cid_time[current][:, history], axis=-1).

Shapes (hardcoded): history (4096,) int64, current (4096,) int64,
cid_time (10000, 10000) float32 -> out (4096, 4096) float32.

Strategy (8 NeuronCores, SPMD; table replicated, state_len sharded):
Each core owns 512 output rows.  Per core:
  A. indirect-DMA row-gather R = table[current_block]  (40KB contiguous rows)
  B. PE-transpose R into HBM staging S3 laid out [rb, t%128, t//128, i]
     so a table-column t of the gathered block is one contiguous 512B row
     (per rb sub-block of 128 i's), written at 40KB/partition line rate.
  C. indirect-DMA row-gather by history (host-transformed indices) ->
     E^T tiles [128 j, 512 i]
  D. exp on ScalarE; column-sums via ones-matmul accumulated in PSUM
  E. PE-transpose back to [i, j] with the softmax normalization fused into
     the PSUM->SBUF evacuation (per-partition reciprocal scalar), then
     contiguous DMA of the [512, 4096] output block.
Softmax skips max-subtraction: inputs are N(0,1) so exp is well-conditioned.
"""

import sys

if "/opt/trn_rl_repo" not in sys.path:
    sys.path.insert(0, "/opt/trn_rl_repo")

import numpy as np

N_CORES = 8
P = 128
NCID = 10000
NTIME = 10000
SEQ = 4096
STATE = 4096
ROWS = STATE // N_CORES          # 512 output rows per core
RB = ROWS // P                   # 4 row-tiles per core
TB = (NTIME + P - 1) // P        # 79 column (time) blocks
TB_LAST_W = NTIME - (TB - 1) * P # 16 wide last block
JB = SEQ // P                    # 32 j-tiles
JQ = JB // 4                     # 8 groups of 4 j-tiles

_prog_cache = {}


def _build_program():
    from concourse import bass, bacc, mybir
    import concourse.tile as tile
    from concourse.masks import make_identity

    f32 = mybir.dt.float32
    i32 = mybir.dt.int32
    Exp = mybir.ActivationFunctionType.Exp

    nc = bacc.Bacc("TRN2", target_bir_lowering=False, debug=False,
                   num_devices=N_CORES)
    table = nc.dram_tensor("table", [NCID, NTIME], f32, kind="ExternalInput").ap()
    cur = nc.dram_tensor("cur", [P, RB], i32, kind="ExternalInput").ap()
    hist = nc.dram_tensor("hist", [P, JB], i32, kind="ExternalInput").ap()
    outp = nc.dram_tensor("outp", [ROWS, SEQ], f32, kind="ExternalOutput").ap()

    with tile.TileContext(nc) as tc:
        with tc.tile_pool(name="dram", bufs=1, space="DRAM") as dpool, \
             tc.tile_pool(name="const", bufs=1) as cpool:
            # S3[p, tb, :] is table-column t = tb*128+p of the gathered
            # block, all 512 i's contiguous (2KB rows for the j-gather)
            S3 = dpool.tile([P, TB, ROWS], f32)  # ~20.7MB HBM staging
            S3_flat = S3[:].rearrange("p t d -> (p t) d")

            ident = cpool.tile([P, P], f32)
            make_identity(nc, ident[:])
            ones = cpool.tile([P, 1], f32)
            nc.vector.memset(ones[:], 1.0)
            cur_sb = cpool.tile([P, RB], i32)
            nc.sync.dma_start(out=cur_sb[:], in_=cur[:, :])
            hist_sb = cpool.tile([P, JB], i32)
            nc.sync.dma_start(out=hist_sb[:], in_=hist[:, :])
            sums_sb = cpool.tile([1, ROWS], f32)
            recip_sb = cpool.tile([P, RB], f32)

            # ---- Phase A/B: gather rows, transpose into S3 staging ----
            with tc.tile_pool(name="rpool", bufs=1) as rpool, \
                 tc.tile_pool(name="spool", bufs=3) as spool, \
                 tc.tile_pool(name="ppool", bufs=6, space="PSUM") as ppool:
                Rt = [rpool.tile([P, NTIME], f32, tag=f"r{rb}", name=f"r{rb}")
                      for rb in range(RB)]  # 4 x 40KB/partition, resident
                for rb in range(RB):
                    nc.gpsimd.indirect_dma_start(
                        out=Rt[rb][:], out_offset=None,
                        in_=table[:, :],
                        in_offset=bass.IndirectOffsetOnAxis(
                            ap=cur_sb[:, rb:rb + 1], axis=0),
                    )
                for tb in range(TB):
                    w = P if tb < TB - 1 else TB_LAST_W
                    pt = ppool.tile([P, ROWS], f32)
                    if w < P:
                        # partial last block: zero so untouched partitions
                        # stay finite
                        nc.vector.memset(pt[:, :], 0.0)
                    for rb in range(RB):
                        nc.tensor.transpose(
                            out=pt[:w, rb * P:(rb + 1) * P],
                            in_=Rt[rb][:, tb * P:tb * P + w],
                            identity=ident[:],
                        )
                    S3sb = spool.tile([P, ROWS], f32)
                    nc.vector.tensor_copy(out=S3sb[:, :], in_=pt[:, :])
                    nc.sync.dma_start(out=S3[:, tb, :], in_=S3sb[:])

            # ---- Phase C/D/E ----
            with tc.tile_pool(name="xpool", bufs=1) as xpool, \
                 tc.tile_pool(name="epool", bufs=6) as epool, \
                 tc.tile_pool(name="opool", bufs=1) as opool, \
                 tc.tile_pool(name="psums", bufs=1, space="PSUM") as pspool, \
                 tc.tile_pool(name="prec", bufs=1, space="PSUM") as precpool, \
                 tc.tile_pool(name="p2", bufs=4, space="PSUM") as p2pool:
                X = xpool.tile([P, JB, RB * P], f32)      # 64KB/partition
                Osb = [opool.tile([P, SEQ], f32, tag=f"o{ib}", name=f"o{ib}")
                       for ib in range(RB)]               # 4 x 16KB/partition
                sums_ps = pspool.tile([1, ROWS], f32)

                for jb in range(JB):
                    E = epool.tile([P, RB * P], f32)
                    nc.gpsimd.indirect_dma_start(
                        out=E[:, :], out_offset=None,
                        in_=S3_flat[:, :],
                        in_offset=bass.IndirectOffsetOnAxis(
                            ap=hist_sb[:, jb:jb + 1], axis=0),
                    )
                    nc.scalar.activation(out=X[:, jb, :], in_=E[:], func=Exp)
                    nc.tensor.matmul(
                        out=sums_ps[:1, :], lhsT=ones[:, :1], rhs=X[:, jb, :],
                        start=(jb == 0), stop=(jb == JB - 1),
                        skip_group_check=True,
                    )

                # reciprocal of column sums, transposed to per-partition form
                nc.vector.tensor_copy(out=sums_sb[:1, :], in_=sums_ps[:1, :])
                rec_ps = precpool.tile([P, RB], f32)
                for ib in range(RB):
                    nc.tensor.transpose(
                        out=rec_ps[:, ib:ib + 1],
                        in_=sums_sb[:1, ib * P:(ib + 1) * P],
                        identity=ident[:1, :1],
                    )
                nc.vector.reciprocal(out=recip_sb[:, :], in_=rec_ps[:, :])

                # transpose back with fused normalize, then store
                for ib in range(RB):
                    for jq in range(JQ):
                        pt2 = p2pool.tile([P, 512], f32)
                        for u in range(4):
                            jb = jq * 4 + u
                            nc.tensor.transpose(
                                out=pt2[:, u * P:(u + 1) * P],
                                in_=X[:, jb, ib * P:(ib + 1) * P],
                                identity=ident[:],
                            )
                        if jq % 2 == 0:
                            nc.vector.tensor_scalar_mul(
                                out=Osb[ib][:, jq * 512:(jq + 1) * 512],
                                in0=pt2[:], scalar1=recip_sb[:, ib:ib + 1])
                        else:
                            nc.scalar.mul(
                                out=Osb[ib][:, jq * 512:(jq + 1) * 512],
                                in_=pt2[:], mul=recip_sb[:, ib:ib + 1])
                    nc.sync.dma_start(out=outp[ib * P:(ib + 1) * P, :],
                                      in_=Osb[ib][:])

    nc.compile()
    return nc


def _get_program():
    if "nc" not in _prog_cache:
        _prog_cache["nc"] = _build_program()
    return _prog_cache["nc"]


def _make_in_maps(history, current, cid_time):
    hist = np.asarray(history).astype(np.int64).ravel()
    curr = np.asarray(current).astype(np.int64).ravel()
    tab = np.ascontiguousarray(np.asarray(cid_time, dtype=np.float32))
    assert hist.shape == (SEQ,) and curr.shape == (STATE,)
    assert tab.shape == (NCID, NTIME)

    # history index j=jb*128+p -> S3_flat row: (h%128)*TB + h//128
    h = hist
    base = ((h % P) * TB + (h // P)).astype(np.int32)
    hist_idx = base.reshape(JB, P).T.copy()  # [p, jb]

    in_maps = []
    for c in range(N_CORES):
        cur_block = curr[c * ROWS:(c + 1) * ROWS].astype(np.int32)
        cur_idx = cur_block.reshape(RB, P).T.copy()  # [p, rb]
        in_maps.append({
            "table": tab,
            "cur": cur_idx,
            "hist": hist_idx,
        })
    return in_maps


def run(history, current, cid_time, trace=False, **trace_kwargs):
    from concourse.bass_utils import run_bass_kernel_spmd

    nc = _get_program()
    in_maps = _make_in_maps(history, current, cid_time)
    res = run_bass_kernel_spmd(nc, in_maps, list(range(N_CORES)),
                               trace=trace, **trace_kwargs)
    out = np.concatenate([res.results[c]["outp"] for c in range(N_CORES)],
                         axis=0)
    return out, res


def kernel(history, current, cid_time):
    out, _ = run(history, current, cid_time, trace=False)
    return out
